# revision 16
# baseline (speedup 1.0000x reference)
"""Bidirectional co-attention kernel for Trainium2 (8 NeuronCores).

Problem: X, Y: (16, 2048, 300) f32.
  S_b = X_b @ Y_b^T                      (2048 x 2048 per batch)
  A1 = softmax_rows(S) @ Y * X
  A2 = softmax_rows(S^T) @ X * Y
  out = concat(A1, A2, axis=1)           -> (16, 4096, 300) f32

Sharding: data-parallel over batch, 2 batches per core, no cross-core comm.

Per-core algorithm (per batch):
  Phase A: S strips [128n x 2048m] on TensorE in float32r (full-rate,
    ~11-bit-mantissa accurate).  D=300 takes 3 K-passes of 128 (k2 plane
    zero-padded).  E_S = exp(S - 100) on ScalarE straight from PSUM into
    bf16 SBUF.  (Fixed shift instead of row-max: scores are N(0, 300) so
    max |S| ~ 95; exp(S-100) never overflows and row maxes are far above
    the underflow cliff.  Normalization cancels the shift exactly.)
  Transposes: E_T = E_S^T via XBAR DMA transposes (dma_start_transpose,
    one [128, 2048] -> [128, 16, 128] call per strip, contiguous dest,
    ~2.3us of sync-queue occupancy each) — zero TensorE/PSUM cost.  All on
    the sync queue at high scheduler priority so they stream during phase
    A as strips complete.  et_big[p, i, m, r] = E^T[m*128+p, i*128+r]:
    direction-1 out-tile i only needs transpose i.
  Phase C: all direction-2 tiles first (lhsT = E_S tiles, no transpose
    dependency, ~32us of PE work), then direction 1 (lhsT = et_big[:, i]).
    The ones column appended to X/Y gives the softmax denominator:
    A1 = O1[:, :300] * X * (1/O1[:, 300]) on VectorE.  Out-DMAs issue from
    the scalar queue (idle during C; sync is busy with transposes).
  Startup: PE warmup matmuls keep the PE busy/p-state ramped while the
    first input slivers stream in.
"""

import numpy as np
import ml_dtypes

B, N, D = 16, 2048, 300
NCORES = 8
BPC = B // NCORES  # batches per core
NT = N // 128  # 16 row-tiles
K2 = 44  # rows in the third K-pass (300 - 256)
GSHIFT = -100.0
DP = 304  # natural-layout tiles padded: col 300 = 1.0 (denominator trick)
NBLK = 512  # moving-dim block for the score matmuls
NWARM = 18  # dummy PE matmuls covering the initial DMA fill + p-state ramp

_BF16 = ml_dtypes.bfloat16

_cache: dict[str, object] = {}


def _build():
    import concourse.bacc as bacc
    import concourse.mybir as mybir
    import concourse.tile as tile

    nc = bacc.Bacc("TRN2", target_bir_lowering=False, debug=False, num_devices=NCORES)

    f32 = mybir.dt.float32
    f32r = mybir.dt.float32r
    bf16 = mybir.dt.bfloat16

    # k0/k1 planes [128, N]; k2 plane rows 0..43 = d 256..299, rows 44..127
    # are zeros (zero-padded K=128 passes measure faster than K=44 ones).
    xt_d = nc.dram_tensor("xt", [BPC, 3, 128, N], f32r, kind="ExternalInput")
    yt_d = nc.dram_tensor("yt", [BPC, 3, 128, N], f32r, kind="ExternalInput")
    xn_d = nc.dram_tensor("xn", [BPC, N, DP], bf16, kind="ExternalInput")
    yn_d = nc.dram_tensor("yn", [BPC, N, DP], bf16, kind="ExternalInput")
    out_d = nc.dram_tensor("out", [BPC, 2 * N, D], f32, kind="ExternalOutput")

    with tile.TileContext(nc) as tc:
        with (
            tc.tile_pool(name="const", bufs=1) as constp,
            tc.tile_pool(name="io", bufs=3) as io,
            tc.tile_pool(name="nat", bufs=1) as nat,
            tc.tile_pool(name="epool", bufs=NT) as epool,
            tc.tile_pool(name="etp", bufs=1) as etp,
            tc.tile_pool(name="stats", bufs=2 * NT) as stats,
            tc.tile_pool(name="abuf", bufs=4) as abuf,
            tc.tile_pool(name="psum", bufs=2, space="PSUM") as psum,
            tc.tile_pool(name="psmall", bufs=4, space="PSUM") as psmall,
        ):
            bias_t = constp.tile([128, 1], f32, name="bias_t")
            nc.vector.memset(bias_t[:], GSHIFT)
            # dummy exp: pulls the one-time ACT_TABLE_LOAD (~2.7us) into the
            # initial DMA ramp instead of the first real softmax strip
            warm = constp.tile([128, 1], f32, name="warm")
            nc.scalar.activation(
                out=warm[:],
                in_=bias_t[:],
                func=mybir.ActivationFunctionType.Exp,
                bias=bias_t[:],
                scale=0.0,
            )
            # PE warmup: keeps the PE busy (and its p-state ramped) while the
            # first input slivers stream in, so real strips run at full rate.
            wsrc = constp.tile([128, NBLK], bf16, name="wsrc")
            nc.vector.memset(wsrc[:], 0.125)
            wps = psum.tile([128, N // 2], f32, tag="strip", name="warm_ps")
            for w in range(NWARM):
                nc.tensor.matmul(
                    wps[:, 0:NBLK],
                    wsrc[:, 0:128],
                    wsrc[:],
                    start=True,
                    stop=True,
                )

            for b in range(BPC):
                # ---- loads.  Batch 0: critical slivers split into 256KB
                # chunks across the sync and scalar queues (a single DMA
                # instruction only sustains ~90GB/s; parallel queues cut the
                # PE start time); yt rest before xt rest (h1 strips need
                # yt[1024:] from ~mid-A, xt[1024:] only at strip 8), rests
                # also split across both queues. ----
                xt_t, yt_t = [], []
                for k in range(3):
                    xk = io.tile([128, N], f32r, tag="xt", name=f"xt{b}_{k}")
                    yk = io.tile([128, N], f32r, tag="yt", name=f"yt{b}_{k}")
                    xt_t.append(xk)
                    yt_t.append(yk)
                if b == 0:
                    # critical slivers in 256KB halves: yt [0:1024] on sync,
                    # xt [0:1024] on scalar.  The scalar-queue DMAs all land
                    # before ~14us — BEFORE the first DMA_TRANSPOSE fires
                    # (~16us): XBAR transposes corrupt when other HWDGE DMA
                    # traffic overlaps them, so everything later stays on
                    # sync, serialized with the transpose stream.
                    for k in range(3):
                        nc.sync.dma_start(
                            yt_t[k][:, 0:512], yt_d.ap()[b, k, :, 0:512]
                        )
                        nc.sync.dma_start(
                            yt_t[k][:, 512:1024], yt_d.ap()[b, k, :, 512:1024]
                        )
                    for k in range(3):
                        nc.scalar.dma_start(
                            xt_t[k][:, 0:512], xt_d.ap()[b, k, :, 0:512]
                        )
                    for k in range(3):
                        nc.scalar.dma_start(
                            xt_t[k][:, 512:1024], xt_d.ap()[b, k, :, 512:1024]
                        )
                    for k in range(3):
                        nc.sync.dma_start(
                            yt_t[k][:, 1024:N], yt_d.ap()[b, k, :, 1024:N]
                        )
                    for k in range(3):
                        nc.sync.dma_start(
                            xt_t[k][:, 1024:N], xt_d.ap()[b, k, :, 1024:N]
                        )
                else:
                    for k in range(3):
                        nc.sync.dma_start(xt_t[k][:], xt_d.ap()[b, k, :, :])
                        nc.sync.dma_start(yt_t[k][:], yt_d.ap()[b, k, :, :])

                # ---- phase A: S strips (half-strip PSUM granularity) + exp.
                # On batch 0 the first 8 strips run h0-major: their h0 halves
                # need only the sliver columns (0:1024), filling the PE while
                # the rest columns stream in; strips 8..15 revert to h-inner
                # so es[0..7] still completes mid-phase. ----
                es_t = [
                    epool.tile([128, N], bf16, tag="e", name=f"es{b}_{i}")
                    for i in range(NT)
                ]
                # E^T store: et_big[p, i, m, r] = E^T[m*128+p, i*128+r].
                # One XBAR DMA transpose per strip i writes the contiguous
                # region et_big[:, i] = [128, 16, 128].
                et_big = etp.tile([128, NT, NT, 128], bf16, tag="et", name=f"et{b}")
                if b == 0:
                    ih_order = (
                        [(i, 0) for i in range(8)]
                        + [(i, 1) for i in range(8)]
                        + [(i, h) for i in range(8, NT) for h in range(2)]
                    )
                else:
                    ih_order = [(i, h) for i in range(NT) for h in range(2)]
                done_h = [0] * NT
                for i, h in ih_order:
                    ei = es_t[i]
                    sp = psum.tile(
                        [128, N // 2], f32, tag="strip", name=f"sp{b}_{i}_{h}"
                    )
                    for k in range(3):
                        lhsT = xt_t[k][:, i * 128 : (i + 1) * 128]
                        for j in range(2):
                            jj = h * 2 + j
                            nc.tensor.matmul(
                                sp[:, j * NBLK : (j + 1) * NBLK],
                                lhsT,
                                yt_t[k][:, jj * NBLK : (jj + 1) * NBLK],
                                start=(k == 0),
                                stop=(k == 2),
                            )
                    nc.scalar.activation(
                        out=ei[:, h * (N // 2) : (h + 1) * (N // 2)],
                        in_=sp[:],
                        func=mybir.ActivationFunctionType.Exp,
                        bias=bias_t[:],
                        scale=1.0,
                    )
                    done_h[i] += 1
                    if done_h[i] == 2:
                        # strip complete: XBAR-transpose it into et_big.
                        # High priority: must beat nat/out DMAs to the sync
                        # queue so the stream keeps pace with phase A.
                        with tc.high_priority():
                            nc.sync.dma_start_transpose(
                                out=et_big[:, i], in_=ei[:]
                            )

                # xn/yn natural-layout tiles: single batched DMA per tensor,
                # on SYNC so the transfers serialize with the DMA_TRANSPOSE
                # stream (concurrent HWDGE traffic corrupts XBAR transposes).
                # First needed at phase C start; issued after the A loop so
                # the scheduler queues them behind the transposes.
                xn_b = nat.tile([128, NT, DP], bf16, tag="xn", name=f"xn{b}")
                yn_b = nat.tile([128, NT, DP], bf16, tag="yn", name=f"yn{b}")
                nc.sync.dma_start(
                    xn_b[:], xn_d.ap()[b].rearrange("(t p) d -> p t d", p=128)
                )
                nc.sync.dma_start(
                    yn_b[:], yn_d.ap()[b].rearrange("(t p) d -> p t d", p=128)
                )

                # ---- phase C: PV matmuls + epilogue.  All dir-2 first (no
                # transpose dependency: ~32us of PE slack for the transpose
                # stream), then dir-1 (out-tile i needs only transpose i). ----
                for which, i in [(1, i) for i in range(NT)] + [
                    (0, i) for i in range(NT)
                ]:
                    rn = yn_b if which == 0 else xn_b
                    mult_n = xn_b if which == 0 else yn_b
                    op = psmall.tile(
                        [128, D + 1], f32, tag="sm", name=f"o{b}_{i}_{which}"
                    )
                    for m in range(NT):
                        if which == 0:
                            lhsT = et_big[:, i, m]
                        else:
                            lhsT = es_t[m][:, i * 128 : (i + 1) * 128]
                        nc.tensor.matmul(
                            op[:],
                            lhsT,
                            rn[:, m, : D + 1],
                            start=(m == 0),
                            stop=(m == NT - 1),
                        )
                    ri = stats.tile(
                        [128, 1], f32, tag="stats", name=f"r{b}_{i}_{which}"
                    )
                    nc.vector.reciprocal(ri[:], op[:, D : D + 1])
                    ai = abuf.tile([128, D], f32, tag="a", name=f"a{b}_{i}_{which}")
                    nc.vector.tensor_mul(ai[:], op[:, :D], mult_n[:, i, :D])
                    nc.vector.tensor_scalar_mul(ai[:], ai[:], ri[:])
                    row0 = which * N + i * 128
                    # out-DMA from the scalar queue: sync is saturated by the
                    # transpose stream, scalar is idle once phase A's exps end
                    nc.scalar.dma_start(out_d.ap()[b, row0 : row0 + 128, :], ai[:])

    nc.compile()
    return nc


def _prep(arr_f32: np.ndarray) -> tuple[np.ndarray, np.ndarray]:
    """arr [Bc, N, D] f32 -> (k-tiled transpose f32 [Bc,3,128,N], zero rows
    above 44 in the k2 plane; bf16 natural [Bc, N, DP], ones column at D)."""
    bc = arr_f32.shape[0]
    at = arr_f32.transpose(0, 2, 1)  # [bc, D, N]
    t = np.zeros((bc, 3, 128, N), np.float32)
    t[:, 0] = at[:, 0:128]
    t[:, 1] = at[:, 128:256]
    t[:, 2, 0:K2] = at[:, 256:300]
    nat = np.zeros((bc, N, DP), _BF16)
    nat[:, :, :D] = arr_f32
    nat[:, :, D] = 1.0
    return np.ascontiguousarray(t), nat


def kernel(X, Y, _trace=False, _trace_kwargs=None):
    from concourse.bass_utils import run_bass_kernel_spmd

    X = np.asarray(X, dtype=np.float32)
    Y = np.asarray(Y, dtype=np.float32)
    assert X.shape == (B, N, D) and Y.shape == (B, N, D)

    if "nc" not in _cache:
        _cache["nc"] = _build()
    nc = _cache["nc"]

    in_maps = []
    for c in range(NCORES):
        sl = slice(c * BPC, (c + 1) * BPC)
        xt, xn = _prep(X[sl])
        yt, yn = _prep(Y[sl])
        in_maps.append({"xt": xt, "yt": yt, "xn": xn, "yn": yn})

    res = run_bass_kernel_spmd(
        nc,
        in_maps,
        core_ids=list(range(NCORES)),
        trace=_trace,
        **(_trace_kwargs or {}),
    )
    _cache["last_results"] = res

    out = np.empty((B, 2 * N, D), np.float32)
    for c in range(NCORES):
        out[c * BPC : (c + 1) * BPC] = res.results[c]["out"]
    return out
